# revision 19
# baseline (speedup 1.0000x reference)
"""GAT layer (nn_GAT_Layer) as a Trainium2 Bass kernel, SPMD over 8 NeuronCores.

Math
----
With E[h,i,j] = e_l[h,i] + e_r[h,j] and A in {0,1}:
  exp(E) = exp(e_l) * exp(e_r)
  denom[h,i] = sum_j exp(E*A) = exp(e_l[h,i]) * (A @ exp(e_r[h]))[i] + (N - deg[i])
  out[h,i,:] = elu( (exp_el/denom)[h,i] * (A @ (exp_er[:,h,None] * HW[:,h,:]))[i] )
where HW = H @ W (per head), deg = A @ 1.

So the only O(N^2) work is one matmul  S = B^T @ A_rows^T  with
B = [G(64) | exp_er(8) | ones(1)]  -> [4096, 73]; everything else is tiny.

Sharding: rows of A are split across the 8 cores (512 rows each). Each core
redundantly computes B (cheap) and its own 512-row epilogue. No collectives.

Host passes A row-blocks pre-transposed so the contraction dim (j) lands on
SBUF partitions, plus a few constant 0/1 selection matrices (pure layout).
"""

import sys

if "/opt/trn_rl_repo" not in sys.path:
    sys.path.insert(0, "/opt/trn_rl_repo")

from contextlib import ExitStack

import numpy as np

import concourse.bass as bass
import concourse.tile as tile
from concourse import bacc, mybir
from concourse.bass_utils import run_bass_kernel_spmd

N, F, HEADS, U = 4096, 128, 8, 8
NCORES = 8
R = N // NCORES            # 512 rows per core
C = HEADS * U              # 64
NB = C + HEADS + 1         # 73 columns of B: G(64) | exp_er(8) | ones(1)
JC = N // 128              # 32 contraction chunks
F32 = mybir.dt.float32
F32R = mybir.dt.float32r

# Big-matmul operand mode: float32r streams at 1 cycle/row (vs 4 for fp32).
# f32r operands must be *produced* as f32r (HW rounds on write); A is 0/1 so
# the rounding is exact for the A operand.
BIG_MM_DTYPE = F32R


def build_bass(reps=1):
    """reps>1 repeats the whole body inside one NEFF (for delta timing)."""
    nc = bacc.Bacc("TRN2", target_bir_lowering=False, debug=True)

    # per-core inputs
    at = nc.declare_dram_parameter("at", [N, R], BIG_MM_DTYPE, isOutput=False)  # A[rows,:].T
    hrt = nc.declare_dram_parameter("hrt", [F, R], F32, isOutput=False)   # H[rows,:].T
    # shared inputs
    ht = nc.declare_dram_parameter("ht", [F, N], F32, isOutput=False)     # H.T
    wfc = nc.declare_dram_parameter("wfc", [F, C], F32, isOutput=False)   # W as [f, h*U+u]
    wt = nc.declare_dram_parameter("wt", [C, F], F32, isOutput=False)     # W as [h*U+u, f]
    alrd = nc.declare_dram_parameter("alrd", [C, 16], F32, isOutput=False)  # blockdiag a_l|a_r
    degc = nc.declare_dram_parameter("degc", [10, 8], F32, isOutput=False)  # deg extractor
    repc = nc.declare_dram_parameter("repc", [8, C], F32, isOutput=False)   # head replicator
    # output (transposed): o[h*U+u, i_local]
    o = nc.declare_dram_parameter("o", [C, R], F32, isOutput=True)

    AF = mybir.ActivationFunctionType
    OP = mybir.AluOpType

    with tile.TileContext(nc) as tc, ExitStack() as ctx:
        consts = ctx.enter_context(tc.tile_pool(name="consts", bufs=1))
        bigp = ctx.enter_context(tc.tile_pool(name="bigp", bufs=1))
        apool = ctx.enter_context(tc.tile_pool(name="apool", bufs=6))
        epool = ctx.enter_context(tc.tile_pool(name="epool", bufs=1))
        bps = ctx.enter_context(tc.tile_pool(name="bps", bufs=2, space="PSUM"))
        spool = ctx.enter_context(tc.tile_pool(name="spool", bufs=1, space="PSUM"))
        mpsum = ctx.enter_context(tc.tile_pool(name="mpsum", bufs=1, space="PSUM"))

        def emit_body():
            # ---- constant / shared loads ----
            # split the 2 MiB H^T load across 8 DMA queues so it doesn't
            # serialize behind one queue (it gates every B-build matmul)
            ht_sb = bigp.tile([F, N], F32, tag="ht_sb")
            for q in range(8):
                nc.sync.dma_start(out=ht_sb[:, q * (N // 8) : (q + 1) * (N // 8)],
                                  in_=ht[:, q * (N // 8) : (q + 1) * (N // 8)])
            hrt_sb = consts.tile([F, R], F32, tag="hrt_sb")
            nc.sync.dma_start(out=hrt_sb, in_=hrt[:, :])
            wt_sb = consts.tile([C, F], F32, tag="wt_sb")
            nc.sync.dma_start(out=wt_sb, in_=wt[:, :])
            alrd_sb = consts.tile([C, 16], F32, tag="alrd_sb")
            nc.sync.dma_start(out=alrd_sb, in_=alrd[:, :])
            # rhs_ext = [ W(f, 64) | WR(f, 8) | zeros(f, 1) ]
            rhs_ext = consts.tile([F, NB], F32, tag="rhs_ext")
            nc.sync.dma_start(out=rhs_ext[:, 0:C], in_=wfc[:, :])
            nc.vector.memset(rhs_ext[:, C + HEADS : NB], 0.0)
            # constants used by epilogue matmuls live at partitions 64.. so
            # their base partition matches the rhs slices they contract with
            degc_sb = consts.tile([128, 8], F32, tag="degc_sb")
            nc.sync.dma_start(out=degc_sb[64:74, :], in_=degc[:, :])
            repc_sb = consts.tile([128, C], F32, tag="repc_sb")
            nc.sync.dma_start(out=repc_sb[64:72, :], in_=repc[:, :])

            # ---- WL | WR : [f, 16] = wt.T @ alrd ----
            wlr_ps = mpsum.tile([F, 16], F32, tag="wlr")
            nc.tensor.matmul(wlr_ps[:, :], lhsT=wt_sb[:, :], rhs=alrd_sb[:, :],
                             start=True, stop=True)
            wl_sb = consts.tile([F, 8], F32, tag="wl_sb")
            nc.vector.tensor_copy(out=wl_sb, in_=wlr_ps[:, 0:8])
            nc.vector.tensor_copy(out=rhs_ext[:, C : C + HEADS], in_=wlr_ps[:, 8:16])

            # ---- e_l for this core's rows: el[h, i] at partitions 64:72 ----
            el_ps = mpsum.tile([128, R], F32, tag="el")
            nc.tensor.matmul(el_ps[64:72, :], lhsT=wl_sb[:, :], rhs=hrt_sb[:, :],
                             start=True, stop=True)
            expel_t = epool.tile([128, R], F32, tag="expel")
            expel = expel_t[64:72, :]
            nc.scalar.activation(out=expel, in_=el_ps[64:72, :], func=AF.Exp)

            # ---- B chunks: b_all[:, c, :] = [G | exp_er | 1] for j-chunk c ----
            b_all = bigp.tile([F, JC, NB], BIG_MM_DTYPE, tag="b_all")
            for c in range(JC):
                pb = bps.tile([128, NB], F32, tag="pb")
                nc.tensor.matmul(pb[:, :], lhsT=ht_sb[:, c * 128 : (c + 1) * 128],
                                 rhs=rhs_ext[:, :], start=True, stop=True)
                # exp over [e_r | 0] -> [exp_er | 1]; one ACT op, DVE copies
                # the f32r view into B
                er_sb = apool.tile([F, HEADS + 1], F32, tag="er_sb")
                nc.scalar.activation(out=er_sb, in_=pb[:, C:NB], func=AF.Exp)
                nc.vector.tensor_copy(out=b_all[:, c, C:NB], in_=er_sb)
                # G = HW * exp_er (broadcast over u within each head)
                nc.vector.tensor_tensor(
                    b_all[:, c, 0:C].rearrange("p (h u) -> p h u", u=U),
                    pb[:, 0:C].rearrange("p (h u) -> p h u", u=U),
                    er_sb[:, 0:HEADS][:, :, None].to_broadcast((F, HEADS, U)),
                    OP.mult,
                )

            # ---- main matmul: S[73, 512] = sum_c B_c^T @ A_c ----
            # A^T streamed as 1 MiB DMAs (4 j-chunks each) for DMA efficiency
            s_ps = spool.tile([NB, R], F32, tag="s")
            at_r = at.rearrange("(cc p) i -> p cc i", p=128)  # [128, 32, 512]
            GRP = 4
            for g in range(JC // GRP):
                a_sb = apool.tile([128, GRP, R], BIG_MM_DTYPE, tag="a")
                nc.sync.dma_start(out=a_sb, in_=at_r[:, g * GRP : (g + 1) * GRP, :])
                for k in range(GRP):
                    c = g * GRP + k
                    nc.tensor.matmul(s_ps[:, :], lhsT=b_all[:, c, :],
                                     rhs=a_sb[:, k, :],
                                     start=(c == 0), stop=(c == JC - 1))

            # ---- epilogue (rows of S: 0:64 = Sg, 64:72 = Se, 72 = deg) ----
            # (4096 - deg) onto partitions 64:72 via a tiny PE matmul
            sed_t = epool.tile([128, R], F32, tag="sed")
            nc.vector.memset(sed_t[64:80, :], 1.0)  # row 73 stays 1.0 (ones row)
            nc.vector.tensor_copy(out=sed_t[64:73, :], in_=s_ps[64:73, :])
            dgc_ps = mpsum.tile([128, R], F32, tag="dgc")
            nc.tensor.matmul(dgc_ps[64:72, :], lhsT=degc_sb[64:74, :],
                             rhs=sed_t[64:74, :], start=True, stop=True)

            # denom = exp_el * Se + (4096 - deg);  ratio = exp_el / denom
            den_t = epool.tile([128, R], F32, tag="den")
            nc.vector.tensor_tensor(den_t[64:72, :], s_ps[64:72, :], expel, OP.mult)
            nc.vector.tensor_add(out=den_t[64:72, :], in0=den_t[64:72, :],
                                 in1=dgc_ps[64:72, :])
            rec_t = epool.tile([128, R], F32, tag="rec")
            nc.vector.reciprocal(out=rec_t[64:72, :], in_=den_t[64:72, :])
            ratio_t = epool.tile([128, R], F32, tag="ratio")
            nc.vector.tensor_mul(out=ratio_t[64:72, :], in0=expel,
                                 in1=rec_t[64:72, :])

            # replicate ratio[h] across the 8 units of each head -> [64, 512]
            rep_ps = mpsum.tile([C, R], F32, tag="rep")
            nc.tensor.matmul(rep_ps[:, :], lhsT=repc_sb[64:72, :],
                             rhs=ratio_t[64:72, :], start=True, stop=True)

            sg_sb = epool.tile([C, R], F32, tag="sg")
            nc.scalar.activation(out=sg_sb, in_=s_ps[0:C, :], func=AF.Copy)
            pre = epool.tile([C, R], F32, tag="pre")
            nc.vector.tensor_mul(out=pre, in0=rep_ps[:, :], in1=sg_sb)

            # elu(x) = relu(x) + exp(min(x, 0)) - 1
            relu_t = epool.tile([C, R], F32, tag="relu_t")
            nc.scalar.activation(out=relu_t, in_=pre, func=AF.Relu)
            mint = epool.tile([C, R], F32, tag="mint")
            nc.vector.tensor_scalar_min(mint, pre, 0.0)
            expm = epool.tile([C, R], F32, tag="expm")
            nc.scalar.activation(out=expm, in_=mint, func=AF.Exp)
            out_sb = epool.tile([C, R], F32, tag="out_sb")
            nc.vector.scalar_tensor_tensor(out_sb, relu_t, -1.0, expm, OP.add, OP.add)

            nc.sync.dma_start(out=o[:, :], in_=out_sb)

        for _ in range(reps):
            emit_body()

    nc.compile()
    return nc


def host_inputs(A, H, W, a_left, a_right):
    """Shard + relayout the full inputs into per-core in_maps (no arithmetic)."""
    A = np.ascontiguousarray(np.asarray(A, dtype=np.float32))
    H = np.ascontiguousarray(np.asarray(H, dtype=np.float32))
    W = np.asarray(W, dtype=np.float32)
    a_left = np.asarray(a_left, dtype=np.float32)
    a_right = np.asarray(a_right, dtype=np.float32)

    ht = np.ascontiguousarray(H.T)                                    # [128, 4096]
    wfc = np.ascontiguousarray(W.transpose(1, 0, 2).reshape(F, C))    # [f, h*U+u]
    wt = np.ascontiguousarray(W.transpose(0, 2, 1).reshape(C, F))     # [h*U+u, f]
    alrd = np.zeros((C, 16), np.float32)
    for h in range(HEADS):
        alrd[h * U : (h + 1) * U, h] = a_left[h]
        alrd[h * U : (h + 1) * U, 8 + h] = a_right[h]
    degc = np.zeros((10, 8), np.float32)
    degc[8, :] = -1.0
    degc[9, :] = float(N)
    repc = np.zeros((8, C), np.float32)
    for h in range(HEADS):
        repc[h, h * U : (h + 1) * U] = 1.0

    shared = {"ht": ht, "wfc": wfc, "wt": wt, "alrd": alrd, "degc": degc,
              "repc": repc}
    in_maps = []
    for k in range(NCORES):
        rows = slice(k * R, (k + 1) * R)
        in_maps.append(dict(
            shared,
            at=np.ascontiguousarray(A[rows, :].T),
            hrt=np.ascontiguousarray(H[rows, :].T),
        ))
    return in_maps


_NC_CACHE = {}


def _get_nc(reps=1):
    if reps not in _NC_CACHE:
        _NC_CACHE[reps] = build_bass(reps)
    return _NC_CACHE[reps]


def run(A, H, W, a_left, a_right, trace=False, **spmd_kwargs):
    nc = _get_nc()
    in_maps = host_inputs(A, H, W, a_left, a_right)
    res = run_bass_kernel_spmd(nc, in_maps, core_ids=list(range(NCORES)),
                               trace=trace, **spmd_kwargs)
    out = np.concatenate([res.results[k]["o"].T for k in range(NCORES)], axis=0)
    return np.ascontiguousarray(out, dtype=np.float32), res


def kernel(A, H, W, a_left, a_right):
    out, _ = run(A, H, W, a_left, a_right, trace=False)
    return out


# revision 24
# speedup vs baseline: 1.7345x; 1.7345x over previous
"""GAT layer (nn_GAT_Layer) as a Trainium2 Bass kernel, SPMD over 8 NeuronCores.

Math
----
With E[h,i,j] = e_l[h,i] + e_r[h,j] and A in {0,1}:
  exp(E) = exp(e_l) * exp(e_r)
  denom[h,i] = sum_j exp(E*A) = exp(e_l[h,i]) * (A @ exp(e_r[h]))[i] + (N - deg[i])
  out[h,i,:] = elu( (exp_el/denom)[h,i] * (A @ (exp_er[:,h,None] * HW[:,h,:]))[i] )
where HW = H @ W (per head), deg = A @ 1.

So the only O(N^2) work is one matmul  S = B^T @ A_rows^T  with
B = [G(64) | exp_er(8) | ones(1)]  -> [4096, 73]; everything else is tiny.

Sharding: rows of A are split across the 8 cores (512 rows each). Each core
redundantly computes B (cheap) and its own 512-row epilogue. No collectives.

Host passes A row-blocks pre-transposed so the contraction dim (j) lands on
SBUF partitions, plus a few constant 0/1 selection matrices (pure layout).
"""

import sys

if "/opt/trn_rl_repo" not in sys.path:
    sys.path.insert(0, "/opt/trn_rl_repo")

from contextlib import ExitStack

import numpy as np

import concourse.bass as bass
import concourse.tile as tile
from concourse import bacc, mybir
from concourse.bass_utils import run_bass_kernel_spmd

N, F, HEADS, U = 4096, 128, 8, 8
NCORES = 8
R = N // NCORES            # 512 rows per core
C = HEADS * U              # 64
NB = C + HEADS + 1         # 73 columns of B: G(64) | exp_er(8) | ones(1)
JC = N // 128              # 32 contraction chunks
F32 = mybir.dt.float32
F32R = mybir.dt.float32r

# Big-matmul mode:
#  "f32r"   - float32r operands, 1 matmul/chunk (1 cyc/row, ~19-bit mantissa),
#             A streamed as 4-byte f32r (exact: A is 0/1)
#  "bf16x2" - A in bf16 (exact), B split into bf16 hi+lo terms, 2 matmuls/chunk
#             (halves the dominant DMA stream; ~fp32 accuracy)
#  "f16"    - A and B in fp16, 1 matmul/chunk (halves DMA; ~1e-3 accuracy)
MODE = "bf16x2"


def build_bass(reps=1, mode=None):
    """reps>1 repeats the whole body inside one NEFF (for delta timing)."""
    mode = mode or MODE
    a_dt = {"f32r": F32R, "bf16x2": mybir.dt.bfloat16, "f16": mybir.dt.float16}[mode]
    b_dt = a_dt
    n_terms = 2 if mode == "bf16x2" else 1

    nc = bacc.Bacc("TRN2", target_bir_lowering=False, debug=True)

    # per-core inputs
    at = nc.declare_dram_parameter("at", [N, R], a_dt, isOutput=False)  # A[rows,:].T
    hrt = nc.declare_dram_parameter("hrt", [F, R], F32, isOutput=False)   # H[rows,:].T
    # shared inputs
    ht = nc.declare_dram_parameter("ht", [F, N], F32, isOutput=False)     # H.T
    wfc = nc.declare_dram_parameter("wfc", [F, C], F32, isOutput=False)   # W as [f, h*U+u]
    wt = nc.declare_dram_parameter("wt", [C, F], F32, isOutput=False)     # W as [h*U+u, f]
    alrd = nc.declare_dram_parameter("alrd", [C, 16], F32, isOutput=False)  # blockdiag a_l|a_r
    degc = nc.declare_dram_parameter("degc", [10, 8], F32, isOutput=False)  # deg extractor
    repc = nc.declare_dram_parameter("repc", [8, C], F32, isOutput=False)   # head replicator
    # output (transposed): o[h*U+u, i_local]
    o = nc.declare_dram_parameter("o", [C, R], F32, isOutput=True)

    AF = mybir.ActivationFunctionType
    OP = mybir.AluOpType

    with tile.TileContext(nc) as tc, ExitStack() as ctx:
        consts = ctx.enter_context(tc.tile_pool(name="consts", bufs=1))
        bigp = ctx.enter_context(tc.tile_pool(name="bigp", bufs=1))
        apool = ctx.enter_context(tc.tile_pool(name="apool", bufs=6))
        epool = ctx.enter_context(tc.tile_pool(name="epool", bufs=1))
        bps = ctx.enter_context(tc.tile_pool(name="bps", bufs=2, space="PSUM"))
        spool = ctx.enter_context(tc.tile_pool(name="spool", bufs=1, space="PSUM"))
        mpsum = ctx.enter_context(tc.tile_pool(name="mpsum", bufs=1, space="PSUM"))

        def emit_body():
            # ---- constant / shared loads ----
            # split the 2 MiB H^T load across 8 DMA queues so it doesn't
            # serialize behind one queue (it gates every B-build matmul)
            ht_sb = bigp.tile([F, N], F32, tag="ht_sb")
            for q in range(8):
                nc.sync.dma_start(out=ht_sb[:, q * (N // 8) : (q + 1) * (N // 8)],
                                  in_=ht[:, q * (N // 8) : (q + 1) * (N // 8)])
            hrt_sb = consts.tile([F, R], F32, tag="hrt_sb")
            nc.sync.dma_start(out=hrt_sb, in_=hrt[:, :])
            wt_sb = consts.tile([C, F], F32, tag="wt_sb")
            nc.sync.dma_start(out=wt_sb, in_=wt[:, :])
            alrd_sb = consts.tile([C, 16], F32, tag="alrd_sb")
            nc.sync.dma_start(out=alrd_sb, in_=alrd[:, :])
            # rhs_ext = [ W(f, 64) | WR(f, 8) | zeros(f, 1) ]
            rhs_ext = consts.tile([F, NB], F32, tag="rhs_ext")
            nc.sync.dma_start(out=rhs_ext[:, 0:C], in_=wfc[:, :])
            nc.vector.memset(rhs_ext[:, C + HEADS : NB], 0.0)
            # constants used by epilogue matmuls live at partitions 64.. so
            # their base partition matches the rhs slices they contract with
            degc_sb = consts.tile([128, 8], F32, tag="degc_sb")
            nc.sync.dma_start(out=degc_sb[64:74, :], in_=degc[:, :])
            repc_sb = consts.tile([128, C], F32, tag="repc_sb")
            nc.sync.dma_start(out=repc_sb[64:72, :], in_=repc[:, :])

            # ---- WL | WR : [f, 16] = wt.T @ alrd ----
            wlr_ps = mpsum.tile([F, 16], F32, tag="wlr")
            nc.tensor.matmul(wlr_ps[:, :], lhsT=wt_sb[:, :], rhs=alrd_sb[:, :],
                             start=True, stop=True)
            wl_sb = consts.tile([F, 8], F32, tag="wl_sb")
            nc.vector.tensor_copy(out=wl_sb, in_=wlr_ps[:, 0:8])
            nc.vector.tensor_copy(out=rhs_ext[:, C : C + HEADS], in_=wlr_ps[:, 8:16])

            # ---- e_l for this core's rows: el[h, i] at partitions 64:72 ----
            el_ps = mpsum.tile([128, R], F32, tag="el")
            nc.tensor.matmul(el_ps[64:72, :], lhsT=wl_sb[:, :], rhs=hrt_sb[:, :],
                             start=True, stop=True)
            expel_t = epool.tile([128, R], F32, tag="expel")
            expel = expel_t[64:72, :]
            nc.scalar.activation(out=expel, in_=el_ps[64:72, :], func=AF.Exp)

            # ---- B chunks: b_all[:, t, c, :] = [G | exp_er | 1] terms ----
            b_all = bigp.tile([F, n_terms, JC, NB], b_dt, tag="b_all")
            for c in range(JC):
                pb = bps.tile([128, NB], F32, tag="pb")
                nc.tensor.matmul(pb[:, :], lhsT=ht_sb[:, c * 128 : (c + 1) * 128],
                                 rhs=rhs_ext[:, :], start=True, stop=True)
                # g_sb = fp32 [G | exp_er | 1] for this chunk
                g_sb = apool.tile([F, NB], F32, tag="g_sb")
                nc.scalar.activation(out=g_sb[:, C:NB], in_=pb[:, C:NB], func=AF.Exp)
                # G = HW * exp_er (broadcast over u within each head)
                nc.vector.tensor_tensor(
                    g_sb[:, 0:C].rearrange("p (h u) -> p h u", u=U),
                    pb[:, 0:C].rearrange("p (h u) -> p h u", u=U),
                    g_sb[:, C : C + HEADS][:, :, None].to_broadcast((F, HEADS, U)),
                    OP.mult,
                )
                # round into the matmul dtype (hi), plus residual term (lo)
                nc.vector.tensor_copy(out=b_all[:, 0, c, :], in_=g_sb)
                if n_terms == 2:
                    nc.vector.tensor_sub(out=b_all[:, 1, c, :], in0=g_sb,
                                         in1=b_all[:, 0, c, :])

            # ---- main matmul: S[73, 512] = sum_c sum_t B_tc^T @ A_c ----
            # A^T streamed as 1 MiB DMAs (4 j-chunks each) for DMA efficiency
            s_ps = spool.tile([NB, R], F32, tag="s")
            at_r = at.rearrange("(cc p) i -> p cc i", p=128)  # [128, 32, 512]
            GRP = 4
            for g in range(JC // GRP):
                a_sb = apool.tile([128, GRP, R], a_dt, tag="a")
                nc.sync.dma_start(out=a_sb, in_=at_r[:, g * GRP : (g + 1) * GRP, :])
                for k in range(GRP):
                    c = g * GRP + k
                    for t in range(n_terms):
                        nc.tensor.matmul(
                            s_ps[:, :], lhsT=b_all[:, t, c, :], rhs=a_sb[:, k, :],
                            start=(c == 0 and t == 0),
                            stop=(c == JC - 1 and t == n_terms - 1))

            # ---- epilogue (rows of S: 0:64 = Sg, 64:72 = Se, 72 = deg) ----
            # (4096 - deg) onto partitions 64:72 via a tiny PE matmul
            sed_t = epool.tile([128, R], F32, tag="sed")
            nc.vector.memset(sed_t[64:80, :], 1.0)  # row 73 stays 1.0 (ones row)
            nc.vector.tensor_copy(out=sed_t[64:73, :], in_=s_ps[64:73, :])
            dgc_ps = mpsum.tile([128, R], F32, tag="dgc")
            nc.tensor.matmul(dgc_ps[64:72, :], lhsT=degc_sb[64:74, :],
                             rhs=sed_t[64:74, :], start=True, stop=True)

            # denom = exp_el * Se + (4096 - deg);  ratio = exp_el / denom
            den_t = epool.tile([128, R], F32, tag="den")
            nc.vector.tensor_tensor(den_t[64:72, :], s_ps[64:72, :], expel, OP.mult)
            nc.vector.tensor_add(out=den_t[64:72, :], in0=den_t[64:72, :],
                                 in1=dgc_ps[64:72, :])
            rec_t = epool.tile([128, R], F32, tag="rec")
            nc.vector.reciprocal(out=rec_t[64:72, :], in_=den_t[64:72, :])
            ratio_t = epool.tile([128, R], F32, tag="ratio")
            nc.vector.tensor_mul(out=ratio_t[64:72, :], in0=expel,
                                 in1=rec_t[64:72, :])

            # replicate ratio[h] across the 8 units of each head -> [64, 512]
            rep_ps = mpsum.tile([C, R], F32, tag="rep")
            nc.tensor.matmul(rep_ps[:, :], lhsT=repc_sb[64:72, :],
                             rhs=ratio_t[64:72, :], start=True, stop=True)

            sg_sb = epool.tile([C, R], F32, tag="sg")
            nc.scalar.activation(out=sg_sb, in_=s_ps[0:C, :], func=AF.Copy)
            pre = epool.tile([C, R], F32, tag="pre")
            nc.vector.tensor_mul(out=pre, in0=rep_ps[:, :], in1=sg_sb)

            # elu(x) = relu(x) + exp(min(x, 0)) - 1
            relu_t = epool.tile([C, R], F32, tag="relu_t")
            nc.scalar.activation(out=relu_t, in_=pre, func=AF.Relu)
            mint = epool.tile([C, R], F32, tag="mint")
            nc.vector.tensor_scalar_min(mint, pre, 0.0)
            expm = epool.tile([C, R], F32, tag="expm")
            nc.scalar.activation(out=expm, in_=mint, func=AF.Exp)
            out_sb = epool.tile([C, R], F32, tag="out_sb")
            nc.vector.scalar_tensor_tensor(out_sb, relu_t, -1.0, expm, OP.add, OP.add)

            nc.sync.dma_start(out=o[:, :], in_=out_sb)

        for _ in range(reps):
            emit_body()

    nc.compile()
    return nc


def host_inputs(A, H, W, a_left, a_right, mode=None):
    """Shard + relayout the full inputs into per-core in_maps (no arithmetic;
    the A cast to bf16/fp16 is exact since A is 0/1)."""
    import ml_dtypes

    mode = mode or MODE
    at_np = {"f32r": np.float32, "bf16x2": ml_dtypes.bfloat16,
             "f16": np.float16}[mode]
    A = np.ascontiguousarray(np.asarray(A, dtype=np.float32))
    H = np.ascontiguousarray(np.asarray(H, dtype=np.float32))
    W = np.asarray(W, dtype=np.float32)
    a_left = np.asarray(a_left, dtype=np.float32)
    a_right = np.asarray(a_right, dtype=np.float32)

    ht = np.ascontiguousarray(H.T)                                    # [128, 4096]
    wfc = np.ascontiguousarray(W.transpose(1, 0, 2).reshape(F, C))    # [f, h*U+u]
    wt = np.ascontiguousarray(W.transpose(0, 2, 1).reshape(C, F))     # [h*U+u, f]
    alrd = np.zeros((C, 16), np.float32)
    for h in range(HEADS):
        alrd[h * U : (h + 1) * U, h] = a_left[h]
        alrd[h * U : (h + 1) * U, 8 + h] = a_right[h]
    degc = np.zeros((10, 8), np.float32)
    degc[8, :] = -1.0
    degc[9, :] = float(N)
    repc = np.zeros((8, C), np.float32)
    for h in range(HEADS):
        repc[h, h * U : (h + 1) * U] = 1.0

    shared = {"ht": ht, "wfc": wfc, "wt": wt, "alrd": alrd, "degc": degc,
              "repc": repc}
    in_maps = []
    for k in range(NCORES):
        rows = slice(k * R, (k + 1) * R)
        in_maps.append(dict(
            shared,
            at=np.ascontiguousarray(A[rows, :].T).astype(at_np),
            hrt=np.ascontiguousarray(H[rows, :].T),
        ))
    return in_maps


_NC_CACHE = {}


def _get_nc(reps=1, mode=None):
    key = (reps, mode or MODE)
    if key not in _NC_CACHE:
        _NC_CACHE[key] = build_bass(reps, mode)
    return _NC_CACHE[key]


def run(A, H, W, a_left, a_right, trace=False, **spmd_kwargs):
    nc = _get_nc()
    in_maps = host_inputs(A, H, W, a_left, a_right)
    res = run_bass_kernel_spmd(nc, in_maps, core_ids=list(range(NCORES)),
                               trace=trace, **spmd_kwargs)
    out = np.concatenate([res.results[k]["o"].T for k in range(NCORES)], axis=0)
    return np.ascontiguousarray(out, dtype=np.float32), res


def kernel(A, H, W, a_left, a_right):
    out, _ = run(A, H, W, a_left, a_right, trace=False)
    return out


# revision 39
# speedup vs baseline: 1.9678x; 1.1345x over previous
"""GAT layer (nn_GAT_Layer) as a Trainium2 Bass kernel, SPMD over 8 NeuronCores.

Math
----
With E[h,i,j] = e_l[h,i] + e_r[h,j] and A in {0,1}:
  exp(E) = exp(e_l) * exp(e_r)
  denom[h,i] = sum_j exp(E*A) = exp(e_l[h,i]) * (A @ exp(e_r[h]))[i] + (N - deg[i])
  out[h,i,:] = elu( (exp_el/denom)[h,i] * (A @ (exp_er[:,h,None] * HW[:,h,:]))[i] )
where HW = H @ W (per head), deg = A @ 1.

So the only O(N^2) work is one matmul  S = B^T @ A_rows^T  with
B = [G(64) | exp_er(8) | ones(1)]  -> [4096, 73]; everything else is tiny.

Sharding: rows of A are split across the 8 cores (512 rows each). Each core
redundantly computes B (cheap) and its own 512-row epilogue. No collectives.

Host passes A row-blocks pre-transposed so the contraction dim (j) lands on
SBUF partitions, plus a few constant 0/1 selection matrices (pure layout).
"""

import sys

if "/opt/trn_rl_repo" not in sys.path:
    sys.path.insert(0, "/opt/trn_rl_repo")

from contextlib import ExitStack

import numpy as np

import concourse.bass as bass
import concourse.tile as tile
from concourse import bacc, mybir
from concourse.bass_utils import run_bass_kernel_spmd

N, F, HEADS, U = 4096, 128, 8, 8
NCORES = 8
R = N // NCORES            # 512 rows per core
C = HEADS * U              # 64
NB = C + HEADS + 1         # 73 columns of B: G(64) | exp_er(8) | ones(1)
JC = N // 128              # 32 contraction chunks
F32 = mybir.dt.float32
F32R = mybir.dt.float32r

# Big-matmul mode:
#  "f32r"   - float32r operands, 1 matmul/chunk (1 cyc/row, ~19-bit mantissa),
#             A streamed as 4-byte f32r (exact: A is 0/1)
#  "bf16x2" - A in bf16 (exact), B split into bf16 hi+lo terms, 2 matmuls/chunk
#             (halves the dominant DMA stream; ~fp32 accuracy)
#  "f16"    - A and B in fp16, 1 matmul/chunk (halves DMA; ~1e-3 accuracy)
#  "f16a8"  - A in fp8e4 (exact), B in fp16, 1 matmul/chunk (quarter A DMA)
MODE = "bf16x2"


def build_bass(reps=1, mode=None):
    """reps>1 repeats the whole body inside one NEFF (for delta timing)."""
    mode = mode or MODE
    a_dt = {"f32r": F32R, "bf16x2": mybir.dt.bfloat16, "f16": mybir.dt.float16,
            "f16a8": mybir.dt.float8e4}[mode]
    b_dt = mybir.dt.float16 if mode == "f16a8" else a_dt
    n_terms = 2 if mode == "bf16x2" else 1

    nc = bacc.Bacc("TRN2", target_bir_lowering=False, debug=True)

    # per-core inputs
    at = nc.declare_dram_parameter("at", [N, R], a_dt, isOutput=False)  # A[rows,:].T
    hrt = nc.declare_dram_parameter("hrt", [F, R], F32, isOutput=False)  # H[rows,:].T
    # shared inputs
    ht = nc.declare_dram_parameter("ht", [F, N], F32, isOutput=False)     # H.T
    wfc = nc.declare_dram_parameter("wfc", [F, C], F32, isOutput=False)   # W as [f, h*U+u]
    wt = nc.declare_dram_parameter("wt", [C, F], F32, isOutput=False)     # W as [h*U+u, f]
    alrd = nc.declare_dram_parameter("alrd", [C, 16], F32, isOutput=False)  # blockdiag a_l|a_r
    degc = nc.declare_dram_parameter("degc", [9, 8], F32, isOutput=False)  # deg extractor
    repc = nc.declare_dram_parameter("repc", [8, C], F32, isOutput=False)   # head replicator
    # output (transposed): o[h*U+u, i_local]
    o = nc.declare_dram_parameter("o", [C, R], F32, isOutput=True)

    AF = mybir.ActivationFunctionType
    OP = mybir.AluOpType

    with tile.TileContext(nc) as tc, ExitStack() as ctx:
        consts = ctx.enter_context(tc.tile_pool(name="consts", bufs=1))
        bigp = ctx.enter_context(tc.tile_pool(name="bigp", bufs=1))
        apool = ctx.enter_context(tc.tile_pool(name="apool", bufs=6))
        epool = ctx.enter_context(tc.tile_pool(name="epool", bufs=1))
        bps = ctx.enter_context(tc.tile_pool(name="bps", bufs=2, space="PSUM"))
        spool = ctx.enter_context(tc.tile_pool(name="spool", bufs=1, space="PSUM"))
        mpsum = ctx.enter_context(tc.tile_pool(name="mpsum", bufs=1, space="PSUM"))

        def emit_body():
            # ---- constant / shared loads ----
            # split the 2 MiB H^T load across 8 DMA queues so it doesn't
            # serialize behind one queue (it gates every B-build matmul)
            ht_sb = bigp.tile([F, N], F32, tag="ht_sb")
            for q in range(8):
                nc.sync.dma_start(out=ht_sb[:, q * (N // 8) : (q + 1) * (N // 8)],
                                  in_=ht[:, q * (N // 8) : (q + 1) * (N // 8)])
            hrt_sb = consts.tile([F, R], F32, tag="hrt_sb")
            nc.sync.dma_start(out=hrt_sb, in_=hrt[:, :])
            wt_sb = consts.tile([C, F], F32, tag="wt_sb")
            nc.sync.dma_start(out=wt_sb, in_=wt[:, :])
            alrd_sb = consts.tile([C, 16], F32, tag="alrd_sb")
            nc.sync.dma_start(out=alrd_sb, in_=alrd[:, :])
            # rhs_ext = [ W(f, 64) | WR(f, 8) | zeros(f, 1) ]
            rhs_ext = consts.tile([F, NB], F32, tag="rhs_ext")
            nc.sync.dma_start(out=rhs_ext[:, 0:C], in_=wfc[:, :])
            nc.vector.memset(rhs_ext[:, C + HEADS : NB], 0.0)
            # constants used by epilogue matmuls live at partitions 64.. so
            # their base partition matches the rhs slices they contract with
            degc_sb = consts.tile([128, 8], F32, tag="degc_sb")
            nc.sync.dma_start(out=degc_sb[64:73, :], in_=degc[:, :])
            repc_sb = consts.tile([128, C], F32, tag="repc_sb")
            nc.sync.dma_start(out=repc_sb[64:72, :], in_=repc[:, :])

            # ---- WL | WR : [f, 16] = wt.T @ alrd ----
            wlr_ps = mpsum.tile([F, 16], F32, tag="wlr")
            nc.tensor.matmul(wlr_ps[:, :], lhsT=wt_sb[:, :], rhs=alrd_sb[:, :],
                             start=True, stop=True)
            wl_sb = consts.tile([F, 8], F32, tag="wl_sb")
            nc.vector.tensor_copy(out=wl_sb, in_=wlr_ps[:, 0:8])
            nc.vector.tensor_copy(out=rhs_ext[:, C : C + HEADS], in_=wlr_ps[:, 8:16])

            # ---- e_l for this core's rows: el[h, i] at partitions 64:72 ----
            el_ps = mpsum.tile([128, R], F32, tag="el")
            nc.tensor.matmul(el_ps[64:72, :], lhsT=wl_sb[:, :], rhs=hrt_sb[:, :],
                             start=True, stop=True)
            expel_t = epool.tile([128, R], F32, tag="expel")
            expel = expel_t[64:72, :]
            nc.scalar.activation(out=expel, in_=el_ps[64:72, :], func=AF.Exp)

            # ---- B chunks: b_all[:, t, c, :] = [G | exp_er | 1] terms ----
            b_all = bigp.tile([F, n_terms, JC, NB], b_dt, tag="b_all")
            for c in range(JC):
                pb = bps.tile([128, NB], F32, tag="pb")
                nc.tensor.matmul(pb[:, :], lhsT=ht_sb[:, c * 128 : (c + 1) * 128],
                                 rhs=rhs_ext[:, :], start=True, stop=True)
                # g_sb = fp32 [G | exp_er | 1] for this chunk
                g_sb = apool.tile([F, NB], F32, tag="g_sb")
                nc.scalar.activation(out=g_sb[:, C:NB], in_=pb[:, C:NB], func=AF.Exp)
                # G = HW * exp_er (broadcast over u within each head)
                nc.vector.tensor_tensor(
                    g_sb[:, 0:C].rearrange("p (h u) -> p h u", u=U),
                    pb[:, 0:C].rearrange("p (h u) -> p h u", u=U),
                    g_sb[:, C : C + HEADS][:, :, None].to_broadcast((F, HEADS, U)),
                    OP.mult,
                )
                # round into the matmul dtype (hi), plus residual term (lo)
                nc.vector.tensor_copy(out=b_all[:, 0, c, :], in_=g_sb)
                if n_terms == 2:
                    nc.vector.tensor_sub(out=b_all[:, 1, c, :], in0=g_sb,
                                         in1=b_all[:, 0, c, :])

            # ---- main matmul: S[73, 512] = sum_c sum_t B_tc^T @ A_c ----
            # A^T streamed as 1 MiB DMAs (4 j-chunks each) for DMA efficiency
            s_ps = spool.tile([NB, R], F32, tag="s")
            at_r = at.rearrange("(cc p) i -> p cc i", p=128)  # [128, 32, 512]
            GRP = 4
            for g in range(JC // GRP):
                a_sb = apool.tile([128, GRP, R], a_dt, tag="a")
                nc.sync.dma_start(out=a_sb, in_=at_r[:, g * GRP : (g + 1) * GRP, :])
                for k in range(GRP):
                    c = g * GRP + k
                    for t in range(n_terms):
                        nc.tensor.matmul(
                            s_ps[:, :], lhsT=b_all[:, t, c, :], rhs=a_sb[:, k, :],
                            start=(c == 0 and t == 0),
                            stop=(c == JC - 1 and t == n_terms - 1))

            # ---- epilogue (rows of S: 0:64 = Sg, 64:72 = Se, 72 = deg) ----
            # -deg onto partitions 64:72 via a tiny PE matmul
            sed_t = epool.tile([128, R], F32, tag="sed")
            nc.vector.tensor_copy(out=sed_t[64:73, :], in_=s_ps[64:73, :])
            dgc_ps = mpsum.tile([128, R], F32, tag="dgc")
            nc.tensor.matmul(dgc_ps[64:72, :], lhsT=degc_sb[64:73, :],
                             rhs=sed_t[64:73, :], start=True, stop=True)

            # denom = exp_el * Se + 4096 - deg;  ratio = exp_el / denom
            den_t = epool.tile([128, R], F32, tag="den")
            nc.vector.tensor_tensor(den_t[64:72, :], s_ps[64:72, :], expel, OP.mult)
            nc.vector.scalar_tensor_tensor(den_t[64:72, :], den_t[64:72, :],
                                           float(N), dgc_ps[64:72, :],
                                           OP.add, OP.add)
            rec_t = epool.tile([128, R], F32, tag="rec")
            nc.vector.reciprocal(out=rec_t[64:72, :], in_=den_t[64:72, :])
            ratio_t = epool.tile([128, R], F32, tag="ratio")
            nc.vector.tensor_mul(out=ratio_t[64:72, :], in0=expel,
                                 in1=rec_t[64:72, :])

            # replicate ratio[h] across the 8 units of each head -> [64, 512]
            rep_ps = mpsum.tile([C, R], F32, tag="rep")
            nc.tensor.matmul(rep_ps[:, :], lhsT=repc_sb[64:72, :],
                             rhs=ratio_t[64:72, :], start=True, stop=True)

            sg_sb = epool.tile([C, R], F32, tag="sg")
            nc.scalar.activation(out=sg_sb, in_=s_ps[0:C, :], func=AF.Copy)
            pre = epool.tile([C, R], F32, tag="pre")
            nc.vector.tensor_mul(out=pre, in0=rep_ps[:, :], in1=sg_sb)

            # elu(x) = relu(x) + exp(min(x, 0)) - 1
            relu_t = epool.tile([C, R], F32, tag="relu_t")
            nc.scalar.activation(out=relu_t, in_=pre, func=AF.Relu)
            mint = epool.tile([C, R], F32, tag="mint")
            nc.vector.tensor_scalar_min(mint, pre, 0.0)
            expm = epool.tile([C, R], F32, tag="expm")
            nc.scalar.activation(out=expm, in_=mint, func=AF.Exp)
            out_sb = epool.tile([C, R], F32, tag="out_sb")
            nc.vector.scalar_tensor_tensor(out_sb, relu_t, -1.0, expm, OP.add, OP.add)

            nc.sync.dma_start(out=o[:, :], in_=out_sb)

        for _ in range(reps):
            emit_body()

    nc.compile()
    return nc


def host_inputs(A, H, W, a_left, a_right, mode=None):
    """Shard + relayout the full inputs into per-core in_maps (no arithmetic;
    the A cast to bf16/fp16 is exact since A is 0/1)."""
    import ml_dtypes

    mode = mode or MODE
    at_np = {"f32r": np.float32, "bf16x2": ml_dtypes.bfloat16,
             "f16": np.float16, "f16a8": ml_dtypes.float8_e4m3}[mode]
    A = np.ascontiguousarray(np.asarray(A, dtype=np.float32))
    H = np.ascontiguousarray(np.asarray(H, dtype=np.float32))
    W = np.asarray(W, dtype=np.float32)
    a_left = np.asarray(a_left, dtype=np.float32)
    a_right = np.asarray(a_right, dtype=np.float32)

    ht = np.ascontiguousarray(H.T)                                    # [128, 4096]
    wfc = np.ascontiguousarray(W.transpose(1, 0, 2).reshape(F, C))    # [f, h*U+u]
    wt = np.ascontiguousarray(W.transpose(0, 2, 1).reshape(C, F))     # [h*U+u, f]
    alrd = np.zeros((C, 16), np.float32)
    for h in range(HEADS):
        alrd[h * U : (h + 1) * U, h] = a_left[h]
        alrd[h * U : (h + 1) * U, 8 + h] = a_right[h]
    degc = np.zeros((9, 8), np.float32)
    degc[8, :] = -1.0
    repc = np.zeros((8, C), np.float32)
    for h in range(HEADS):
        repc[h, h * U : (h + 1) * U] = 1.0

    shared = {"ht": ht, "wfc": wfc, "wt": wt, "alrd": alrd, "degc": degc,
              "repc": repc}
    in_maps = []
    for k in range(NCORES):
        rows = slice(k * R, (k + 1) * R)
        in_maps.append(dict(
            shared,
            at=np.ascontiguousarray(A[rows, :].T).astype(at_np),
            hrt=np.ascontiguousarray(H[rows, :].T),
        ))
    return in_maps


_NC_CACHE = {}


def _get_nc(reps=1, mode=None):
    key = (reps, mode or MODE)
    if key not in _NC_CACHE:
        _NC_CACHE[key] = build_bass(reps, mode)
    return _NC_CACHE[key]


def run(A, H, W, a_left, a_right, trace=False, **spmd_kwargs):
    nc = _get_nc()
    in_maps = host_inputs(A, H, W, a_left, a_right)
    res = run_bass_kernel_spmd(nc, in_maps, core_ids=list(range(NCORES)),
                               trace=trace, **spmd_kwargs)
    out = np.concatenate([res.results[k]["o"].T for k in range(NCORES)], axis=0)
    return np.ascontiguousarray(out, dtype=np.float32), res


def kernel(A, H, W, a_left, a_right):
    out, _ = run(A, H, W, a_left, a_right, trace=False)
    return out


# revision 48
# speedup vs baseline: 2.0055x; 1.0191x over previous
"""GAT layer (nn_GAT_Layer) as a Trainium2 Bass kernel, SPMD over 8 NeuronCores.

Math
----
With E[h,i,j] = e_l[h,i] + e_r[h,j] and A in {0,1}:
  exp(E) = exp(e_l) * exp(e_r)
  denom[h,i] = sum_j exp(E*A) = exp(e_l[h,i]) * (A @ exp(e_r[h]))[i] + (N - deg[i])
  out[h,i,:] = elu( (exp_el/denom)[h,i] * (A @ (exp_er[:,h,None] * HW[:,h,:]))[i] )
where HW = H @ W (per head), deg = A @ 1.

So the only O(N^2) work is one matmul  S = B^T @ A_rows^T  with
B = [G(64) | exp_er(8) | ones(1)]  -> [4096, 73]; everything else is tiny.

Sharding: rows of A are split across the 8 cores (512 rows each). Each core
redundantly computes B (cheap) and its own 512-row epilogue. No collectives.

Host passes A row-blocks pre-transposed so the contraction dim (j) lands on
SBUF partitions, plus a few constant 0/1 selection matrices (pure layout).
"""

import sys

if "/opt/trn_rl_repo" not in sys.path:
    sys.path.insert(0, "/opt/trn_rl_repo")

from contextlib import ExitStack

import numpy as np

import concourse.bass as bass
import concourse.tile as tile
from concourse import bacc, mybir
from concourse.bass_utils import run_bass_kernel_spmd

N, F, HEADS, U = 4096, 128, 8, 8
NCORES = 8
R = N // NCORES            # 512 rows per core
C = HEADS * U              # 64
NB = C + HEADS + 1         # 73 columns of B: G(64) | exp_er(8) | ones(1)
JC = N // 128              # 32 contraction chunks
F32 = mybir.dt.float32
F32R = mybir.dt.float32r

# Big-matmul mode. Fields: a (A dtype), b (B dtype), terms (1 = single,
# 2 = hi+lo residual split of B), build (dtype of H^T / W operands of the
# B-build matmuls), el (dtype of the e_l matmul operands).
# A casts are exact (A is 0/1 so bf16/fp16/fp8e4 represent it exactly).
BF16, F16, F8 = mybir.dt.bfloat16, mybir.dt.float16, mybir.dt.float8e4
MODES = {
    "f32r":    dict(a=F32R, b=F32R, terms=1, build=F32, el=F32),
    "bf16x2":  dict(a=BF16, b=BF16, terms=2, build=F32, el=F32),
    "f16":     dict(a=F16, b=F16, terms=1, build=F32, el=F32),
    "f16a8":   dict(a=F8, b=F16, terms=1, build=F32, el=F32),
    "f16f":    dict(a=F16, b=F16, terms=1, build=F16, el=F16),
    "f16fa8":  dict(a=F8, b=F16, terms=1, build=F16, el=F16),
    "bf16x2f": dict(a=BF16, b=BF16, terms=2, build=F16, el=F16),
}
MODE = "bf16x2"


def build_bass(reps=1, mode=None):
    """reps>1 repeats the whole body inside one NEFF (for delta timing)."""
    mode = mode or MODE
    cfg = MODES[mode]
    a_dt, b_dt, n_terms = cfg["a"], cfg["b"], cfg["terms"]
    build_dt, el_dt = cfg["build"], cfg["el"]

    nc = bacc.Bacc("TRN2", target_bir_lowering=False, debug=True)

    # per-core inputs
    at = nc.declare_dram_parameter("at", [N, R], a_dt, isOutput=False)  # A[rows,:].T
    hrt = nc.declare_dram_parameter("hrt", [F, R], el_dt, isOutput=False)  # H[rows,:].T
    # shared inputs
    ht = nc.declare_dram_parameter("ht", [F, N], build_dt, isOutput=False)  # H.T
    wfc = nc.declare_dram_parameter("wfc", [F, C], build_dt, isOutput=False)  # W as [f, h*U+u]
    wt = nc.declare_dram_parameter("wt", [C, F], F32, isOutput=False)     # W as [h*U+u, f]
    alrd = nc.declare_dram_parameter("alrd", [C, 16], F32, isOutput=False)  # blockdiag a_l|a_r
    degc = nc.declare_dram_parameter("degc", [9, 8], F32, isOutput=False)  # deg extractor
    repc = nc.declare_dram_parameter("repc", [8, C], F32, isOutput=False)   # head replicator
    # output (transposed): o[h*U+u, i_local]
    o = nc.declare_dram_parameter("o", [C, R], F32, isOutput=True)

    AF = mybir.ActivationFunctionType
    OP = mybir.AluOpType

    with tile.TileContext(nc) as tc, ExitStack() as ctx:
        consts = ctx.enter_context(tc.tile_pool(name="consts", bufs=1))
        bigp = ctx.enter_context(tc.tile_pool(name="bigp", bufs=1))
        apool = ctx.enter_context(tc.tile_pool(name="apool", bufs=6))
        epool = ctx.enter_context(tc.tile_pool(name="epool", bufs=1))
        bps = ctx.enter_context(tc.tile_pool(name="bps", bufs=2, space="PSUM"))
        spool = ctx.enter_context(tc.tile_pool(name="spool", bufs=1, space="PSUM"))
        mpsum = ctx.enter_context(tc.tile_pool(name="mpsum", bufs=1, space="PSUM"))

        def emit_body():
            # ---- constant / shared loads ----
            # split the 2 MiB H^T load across 8 DMA queues so it doesn't
            # serialize behind one queue (it gates every B-build matmul)
            ht_sb = bigp.tile([F, N], build_dt, tag="ht_sb")
            for q in range(8):
                nc.sync.dma_start(out=ht_sb[:, q * (N // 8) : (q + 1) * (N // 8)],
                                  in_=ht[:, q * (N // 8) : (q + 1) * (N // 8)])
            hrt_sb = consts.tile([F, R], el_dt, tag="hrt_sb")
            nc.sync.dma_start(out=hrt_sb, in_=hrt[:, :])
            wt_sb = consts.tile([C, F], F32, tag="wt_sb")
            nc.sync.dma_start(out=wt_sb, in_=wt[:, :])
            alrd_sb = consts.tile([C, 16], F32, tag="alrd_sb")
            nc.sync.dma_start(out=alrd_sb, in_=alrd[:, :])
            # rhs_ext = [ W(f, 64) | WR(f, 8) | zeros(f, 1) ]
            rhs_ext = consts.tile([F, NB], build_dt, tag="rhs_ext")
            nc.sync.dma_start(out=rhs_ext[:, 0:C], in_=wfc[:, :])
            nc.vector.memset(rhs_ext[:, C + HEADS : NB], 0.0)
            # constants used by epilogue matmuls live at partitions 64.. so
            # their base partition matches the rhs slices they contract with
            degc_sb = consts.tile([128, 8], F32, tag="degc_sb")
            nc.sync.dma_start(out=degc_sb[64:73, :], in_=degc[:, :])
            repc_sb = consts.tile([128, C], F32, tag="repc_sb")
            nc.sync.dma_start(out=repc_sb[64:72, :], in_=repc[:, :])

            # ---- WL | WR : [f, 16] = wt.T @ alrd ----
            wlr_ps = mpsum.tile([F, 16], F32, tag="wlr")
            nc.tensor.matmul(wlr_ps[:, :], lhsT=wt_sb[:, :], rhs=alrd_sb[:, :],
                             start=True, stop=True)
            wl_sb = consts.tile([F, 8], el_dt, tag="wl_sb")
            nc.vector.tensor_copy(out=wl_sb, in_=wlr_ps[:, 0:8])
            nc.vector.tensor_copy(out=rhs_ext[:, C : C + HEADS], in_=wlr_ps[:, 8:16])

            # ---- e_l for this core's rows: el[h, i] at partitions 64:72 ----
            el_ps = mpsum.tile([128, R], F32, tag="el")
            nc.tensor.matmul(el_ps[64:72, :], lhsT=wl_sb[:, :], rhs=hrt_sb[:, :],
                             start=True, stop=True)
            expel_t = epool.tile([128, R], F32, tag="expel")
            expel = expel_t[64:72, :]
            nc.scalar.activation(out=expel, in_=el_ps[64:72, :], func=AF.Exp)

            # ---- B chunks: b_all[:, t, c, :] = [G | exp_er | 1] terms ----
            b_all = bigp.tile([F, n_terms, JC, NB], b_dt, tag="b_all")
            for c in range(JC):
                pb = bps.tile([128, NB], F32, tag="pb")
                nc.tensor.matmul(pb[:, :], lhsT=ht_sb[:, c * 128 : (c + 1) * 128],
                                 rhs=rhs_ext[:, :], start=True, stop=True)
                # g_sb = fp32 [G | exp_er | 1] for this chunk
                g_sb = apool.tile([F, NB], F32, tag="g_sb")
                nc.scalar.activation(out=g_sb[:, C:NB], in_=pb[:, C:NB], func=AF.Exp)
                # G = HW * exp_er (broadcast over u within each head)
                nc.vector.tensor_tensor(
                    g_sb[:, 0:C].rearrange("p (h u) -> p h u", u=U),
                    pb[:, 0:C].rearrange("p (h u) -> p h u", u=U),
                    g_sb[:, C : C + HEADS][:, :, None].to_broadcast((F, HEADS, U)),
                    OP.mult,
                )
                # round into the matmul dtype (hi), plus residual term (lo)
                nc.vector.tensor_copy(out=b_all[:, 0, c, :], in_=g_sb)
                if n_terms == 2:
                    nc.vector.tensor_sub(out=b_all[:, 1, c, :], in0=g_sb,
                                         in1=b_all[:, 0, c, :])

            # ---- main matmul: S[73, 512] = sum_c sum_t B_tc^T @ A_c ----
            # A^T streamed as 1 MiB DMAs (4 j-chunks each) for DMA efficiency
            s_ps = spool.tile([NB, R], F32, tag="s")
            at_r = at.rearrange("(cc p) i -> p cc i", p=128)  # [128, 32, 512]
            GRP = 4
            for g in range(JC // GRP):
                a_sb = apool.tile([128, GRP, R], a_dt, tag="a")
                nc.sync.dma_start(out=a_sb, in_=at_r[:, g * GRP : (g + 1) * GRP, :])
                for k in range(GRP):
                    c = g * GRP + k
                    for t in range(n_terms):
                        nc.tensor.matmul(
                            s_ps[:, :], lhsT=b_all[:, t, c, :], rhs=a_sb[:, k, :],
                            start=(c == 0 and t == 0),
                            stop=(c == JC - 1 and t == n_terms - 1))

            # ---- epilogue (rows of S: 0:64 = Sg, 64:72 = Se, 72 = deg) ----
            # -deg onto partitions 64:72 via a tiny PE matmul
            sed_t = epool.tile([128, R], F32, tag="sed")
            nc.vector.tensor_copy(out=sed_t[64:73, :], in_=s_ps[64:73, :])
            dgc_ps = mpsum.tile([128, R], F32, tag="dgc")
            nc.tensor.matmul(dgc_ps[64:72, :], lhsT=degc_sb[64:73, :],
                             rhs=sed_t[64:73, :], start=True, stop=True)

            # denom = exp_el * Se + 4096 - deg;  ratio = exp_el / denom
            den_t = epool.tile([128, R], F32, tag="den")
            nc.vector.tensor_tensor(den_t[64:72, :], s_ps[64:72, :], expel, OP.mult)
            nc.vector.scalar_tensor_tensor(den_t[64:72, :], den_t[64:72, :],
                                           float(N), dgc_ps[64:72, :],
                                           OP.add, OP.add)
            rec_t = epool.tile([128, R], F32, tag="rec")
            nc.vector.reciprocal(out=rec_t[64:72, :], in_=den_t[64:72, :])
            ratio_t = epool.tile([128, R], F32, tag="ratio")
            nc.vector.tensor_mul(out=ratio_t[64:72, :], in0=expel,
                                 in1=rec_t[64:72, :])

            # replicate ratio[h] across the 8 units of each head -> [64, 512]
            rep_ps = mpsum.tile([C, R], F32, tag="rep")
            nc.tensor.matmul(rep_ps[:, :], lhsT=repc_sb[64:72, :],
                             rhs=ratio_t[64:72, :], start=True, stop=True)

            sg_sb = epool.tile([C, R], F32, tag="sg")
            nc.scalar.activation(out=sg_sb, in_=s_ps[0:C, :], func=AF.Copy)
            pre = epool.tile([C, R], F32, tag="pre")
            nc.vector.tensor_mul(out=pre, in0=rep_ps[:, :], in1=sg_sb)

            # elu(x) = relu(x) + exp(min(x, 0)) - 1
            relu_t = epool.tile([C, R], F32, tag="relu_t")
            nc.scalar.activation(out=relu_t, in_=pre, func=AF.Relu)
            mint = epool.tile([C, R], F32, tag="mint")
            nc.vector.tensor_scalar_min(mint, pre, 0.0)
            expm = epool.tile([C, R], F32, tag="expm")
            nc.scalar.activation(out=expm, in_=mint, func=AF.Exp)
            out_sb = epool.tile([C, R], F32, tag="out_sb")
            nc.vector.scalar_tensor_tensor(out_sb, relu_t, -1.0, expm, OP.add, OP.add)

            nc.sync.dma_start(out=o[:, :], in_=out_sb)

        for _ in range(reps):
            emit_body()

    nc.compile()
    return nc


def host_inputs(A, H, W, a_left, a_right, mode=None):
    """Shard + relayout the full inputs into per-core in_maps (no arithmetic;
    the A cast to bf16/fp16 is exact since A is 0/1)."""
    mode = mode or MODE
    cfg = MODES[mode]
    at_np = mybir.dt.np(cfg["a"])
    build_np = mybir.dt.np(cfg["build"])
    el_np = mybir.dt.np(cfg["el"])
    A = np.ascontiguousarray(np.asarray(A, dtype=np.float32))
    H = np.ascontiguousarray(np.asarray(H, dtype=np.float32))
    W = np.asarray(W, dtype=np.float32)
    a_left = np.asarray(a_left, dtype=np.float32)
    a_right = np.asarray(a_right, dtype=np.float32)

    ht = np.ascontiguousarray(H.T).astype(build_np)                   # [128, 4096]
    wfc = np.ascontiguousarray(W.transpose(1, 0, 2).reshape(F, C)).astype(build_np)
    wt = np.ascontiguousarray(W.transpose(0, 2, 1).reshape(C, F))     # [h*U+u, f]
    alrd = np.zeros((C, 16), np.float32)
    for h in range(HEADS):
        alrd[h * U : (h + 1) * U, h] = a_left[h]
        alrd[h * U : (h + 1) * U, 8 + h] = a_right[h]
    degc = np.zeros((9, 8), np.float32)
    degc[8, :] = -1.0
    repc = np.zeros((8, C), np.float32)
    for h in range(HEADS):
        repc[h, h * U : (h + 1) * U] = 1.0

    shared = {"ht": ht, "wfc": wfc, "wt": wt, "alrd": alrd, "degc": degc,
              "repc": repc}
    in_maps = []
    for k in range(NCORES):
        rows = slice(k * R, (k + 1) * R)
        in_maps.append(dict(
            shared,
            at=np.ascontiguousarray(A[rows, :].T).astype(at_np),
            hrt=np.ascontiguousarray(H[rows, :].T).astype(el_np),
        ))
    return in_maps


_NC_CACHE = {}


def _get_nc(reps=1, mode=None):
    key = (reps, mode or MODE)
    if key not in _NC_CACHE:
        _NC_CACHE[key] = build_bass(reps, mode)
    return _NC_CACHE[key]


def run(A, H, W, a_left, a_right, trace=False, **spmd_kwargs):
    nc = _get_nc()
    in_maps = host_inputs(A, H, W, a_left, a_right)
    res = run_bass_kernel_spmd(nc, in_maps, core_ids=list(range(NCORES)),
                               trace=trace, **spmd_kwargs)
    out = np.concatenate([res.results[k]["o"].T for k in range(NCORES)], axis=0)
    return np.ascontiguousarray(out, dtype=np.float32), res


def kernel(A, H, W, a_left, a_right):
    out, _ = run(A, H, W, a_left, a_right, trace=False)
    return out
